# revision 1
# baseline (speedup 1.0000x reference)
"""Trainium2 Bass kernel: gradient of the EnergyAttention scalar energy.

reference:
    q = einsum('bqd,hzd->bqhz', g, wq); k = einsum('bkd,hzd->bkhz', g, wk)
    scores = einsum('bqhz,bkhz->bhqk', q, k)
    E = -(logsumexp(BETA*scores, -1)/BETA).sum() + POS_SCALE*(g*pos).sum()
    out = dE/dg

Math: with P = softmax(BETA*scores) per (b,h,q):
    out[b] = -sum_h [ (P@K) @ wq_h + (P.T@Qn) @ wk_h ] + POS_SCALE*pos
where Qn = diag(1/Z) Q (row-normalized by the softmax partition Z).

Sharding: 8 cores; core c handles batch b=c//4 and heads 4*(c%4)..4*(c%4)+3
(two head-pairs packed into the 128-partition dim).  All device I/O is fp16
(full accuracy margin: values here are small, so fp16 beats bf16 ~8x) and
all matmuls run with fp16 operands at full PE rate, f32 PSUM accumulation.

Two I/O layouts (COMM flag):
  COMM=True:  each core uploads only its 256-row chunk of x plus its own 4
      heads of wq/wk; x is AllGather'd on-device inside each 4-core batch
      group, the positive partials are ReduceScatter'd on-device, and the
      negation + positional term are fused on DVE so each core downloads
      only its own 256 output rows (host unshard is a pure reshape).
  COMM=False (default): full-x upload per core, each core returns its full
      [S, D] positive partial, host sums 4 partials per batch and applies
      the positional term.  Used because this container's axon/fake_nrt
      runtime emulates collectives at ~2.4 ms each (measured), dwarfing the
      ~135 us compute; on native NRT flip COMM=True.

Per-core structure (fused pipeline; P tiles are consumed by the transposed
projection matmuls as soon as they are exp'd, so no full [S,S] matrix is
ever materialized in f32):
  prep:   PE transposes x -> G^T tiles; wq/wk natural -> W^T tiles
  proj:   QT2/KT2 [z2, s] = (W G^T) via fp16 matmuls (d contracted, 512-wide
          PSUM chunks)
  trans:  Qraw/K2n [s, z2] via PE transpose-mode
  loop i: scores blocks (row-tiled K=64 pairs) -> one wide exp on ACT per
          head (fp16 out, fused row-sum accum for Z) -> dK^T += (Q/Z)^T @ P
          -> scoresT blocks + unnormalized P^T exp
  burst:  dQ^T += K-block @ PT-block (deferred one pair), then Z-rescaled
          via a partition-broadcast of 1/Z
  out:    grad = sum_pairs dQT^T wq + dKT^T wk

This runtime (axon/fake_nrt) executes NEFFs at ~1.4 us per engine
instruction and ~2.3 ms per collective, so the structure above minimizes
INSTRUCTION COUNT, not engine cycles: all device I/O goes through 3 merged
dma_starts (x load, weights load, output store), every exp is 1024 wide,
and the transposed-P blocks come from a scoresT matmul recompute (32+32
instructions/pair) rather than 128 PE transposes/pair.  On native NRT the
cycle-optimal variants (per-chunk exps, PE-transposed P^T, COMM=True) are
worth revisiting.
"""

import numpy as np

B = 2
S = 1024
D = 1024
NH = 16
Z = 64
BETA = 1.0 / np.sqrt(np.float32(Z))
POS_SCALE = 0.001
N_CORES = 8
HPC = 4           # heads per core
NPAIR = 2         # head pairs per core
ND = D // 128     # 8 d-tiles
NQ = S // 128     # 8 q/k blocks
NCH = S // 512    # 2 moving-dim chunks
CHUNK = S // 4    # 256 rows of x uploaded / of out downloaded per core

_CACHE = {}
# On-device AllGather/ReduceScatter halve the wire traffic and would be the
# right choice on native NRT, but under the axon/fake_nrt runtime each
# collective costs ~2.4 ms (software emulated), so the default ships the
# no-collective variant: full-x upload per core, host-side combine.
COMM = False


def build_nc(reps=1, comm=COMM):
    """Build the (SPMD, identical-per-core) Bass program.

    reps>1 repeats the whole computation (idempotent) inside one NEFF --
    used to measure steady-state per-execution time as a marginal cost.
    comm=False swaps the on-device AllGather/ReduceScatter for full-x
    uploads and host-side combining (for runtimes with slow collectives)."""
    from contextlib import ExitStack

    import concourse.mybir as mybir
    import concourse.tile as tile
    from concourse import bacc
    from concourse.masks import make_identity

    F32 = mybir.dt.float32
    F16 = mybir.dt.float16
    MUL = mybir.AluOpType.mult
    ADD = mybir.AluOpType.add
    BYP = mybir.AluOpType.bypass
    EXP = mybir.ActivationFunctionType.Exp
    CPY = mybir.ActivationFunctionType.Copy
    GROUPS = [[0, 1, 2, 3], [4, 5, 6, 7]]

    nc = bacc.Bacc(
        "TRN2",
        target_bir_lowering=False,
        debug=False,
        enable_asserts=False,
        num_devices=N_CORES,
    )

    xc = nc.dram_tensor("xc", [CHUNK if comm else S, D], F16, kind="ExternalInput").ap()
    wn_in = nc.dram_tensor("wn", [2 * NPAIR * 128, D], F16, kind="ExternalInput").ap()
    if comm:
        posc = nc.dram_tensor("posc", [128, 2], F32, kind="ExternalInput").ap()
    gout = nc.dram_tensor("gout", [CHUNK if comm else S, D], F16, kind="ExternalOutput").ap()

    with tile.TileContext(nc) as tc, ExitStack() as ctx:
        sb1 = ctx.enter_context(tc.tile_pool(name="sb1", bufs=1))
        sb2 = ctx.enter_context(tc.tile_pool(name="sb2", bufs=2))
        sb3 = ctx.enter_context(tc.tile_pool(name="sb3", bufs=3))
        sb4 = ctx.enter_context(tc.tile_pool(name="sb4", bufs=4))
        pp = ctx.enter_context(tc.tile_pool(name="pp", bufs=8))
        dram = ctx.enter_context(tc.tile_pool(name="dram", bufs=1, space="DRAM"))
        # PSUM: "sc" 2x[128,1024] (4 banks) shared by proj/transposes/scores/
        # P-transposes/outproj; "d" 2x[128,1024] (4 banks) for the dK then dQ
        # accumulators (each head's accumulator owns a whole tile so each
        # has_written group has its own banks) -> exactly 8 banks.
        ps_sc = ctx.enter_context(tc.tile_pool(name="ps_sc", bufs=2, space="PSUM"))
        ps_d = ctx.enter_context(tc.tile_pool(name="ps_d", bufs=2, space="PSUM"))

        ident = sb1.tile([128, 128], F32, tag="ident")
        make_identity(nc, ident[:])
        ident_h = sb1.tile([128, 128], F16, tag="ident_h")
        nc.vector.tensor_copy(ident_h[:], ident[:])

        for _rep in range(reps):
            # ---- x AllGather (own 4-core batch group) -----------------------
            if comm:
                xin_b = dram.tile([CHUNK, D], F16, tag="xin_b", name=f"xin{_rep}")
                xg = dram.tile([S, D], F16, tag="xg", name=f"xg{_rep}")
                nc.sync.dma_start(xin_b[:], xc[:])
                nc.gpsimd.collective_compute(
                    "AllGather", BYP, replica_groups=GROUPS,
                    ins=[xin_b[:]], outs=[xg[:]],
                )
            else:
                xg = xc

            # ---- weights: ONE natural load + W^T via PE transposes ----------
            # wn_all [z2, (qk, pair, d)] fp16: wq pairs at cols 0..2D, wk
            # pairs at 2D..4D (single dma_start -- per-DMA overhead in this
            # runtime is ~134us, so every merged DMA counts)
            wn_all = sb1.tile([128, 2 * NPAIR * D], F16, tag="wn_all")
            nc.sync.dma_start(
                wn_all[:].rearrange("p (b d) -> p b d", b=2 * NPAIR),
                wn_in[:].rearrange("(b p) d -> p b d", p=128),
            )
            # wtq/wtk [d_in_tile, (pair, dt, z2)] fp16 via transposes
            wtq = sb1.tile([128, NPAIR * ND * 128], F16, tag="wtq")
            wtk = sb1.tile([128, NPAIR * ND * 128], F16, tag="wtk")
            for wbase, wt in ((0, wtq), (NPAIR * D, wtk)):
                for p in range(NPAIR):
                    ps = ps_sc.tile([128, S], F16, tag="ps_sc",
                                    name=f"wt{_rep}_{wt.tensor.name}_{p}")
                    for dt in range(ND):
                        nc.tensor.transpose(
                            ps[:, dt * 128 : (dt + 1) * 128],
                            wn_all[:, wbase + p * D + dt * 128 : wbase + p * D + (dt + 1) * 128],
                            ident_h[:],
                        )
                    nc.vector.tensor_copy(
                        wt[:, p * ND * 128 : (p + 1) * ND * 128], ps[:]
                    )

            # ---- G^T tiles from x via PE transposes (ONE merged load) -------
            xs_all = sb1.tile([128, NQ * D], F16, tag="xs_all")  # [s_local, (i, d)]
            nc.sync.dma_start(
                xs_all[:].rearrange("p (b d) -> p b d", b=NQ),
                xg[:].rearrange("(b p) d -> p b d", p=128),
            )
            gt = sb1.tile([128, ND * S], F16, tag="gt")  # [d_in_tile, (dt, s)]
            gt_r = gt[:].rearrange("p (d s) -> p d s", d=ND)
            for i in range(NQ):
                ps = ps_sc.tile([128, S], F16, tag="ps_sc", name=f"xt{_rep}_{i}")
                for dt in range(ND):
                    nc.tensor.transpose(
                        ps[:, dt * 128 : (dt + 1) * 128],
                        xs_all[:, i * D + dt * 128 : i * D + (dt + 1) * 128],
                        ident_h[:],
                    )
                nc.vector.tensor_copy(
                    gt_r[:, :, i * 128 : (i + 1) * 128],
                    ps[:].rearrange("p (d c) -> p d c", d=ND),
                )

            # persistent across pairs
            dqt2 = sb1.tile([128, NPAIR * S], F16, tag="dqt2")  # [z2, (pair, q)]
            dkt2 = sb1.tile([128, NPAIR * S], F16, tag="dkt2")  # [z2, (pair, k)]
            zrowA = sb1.tile([1, S], F32, tag="zrowA")
            zrowB = sb1.tile([1, S], F32, tag="zrowB")
            ztsb = sb1.tile([16, 128], F32, tag="ztsb")

            pending_dq = []

            def emit_dq_burst():
                """dQ^T(unnorm) += K_i^T PT_i over all blocks, then Z-scale."""
                if not pending_dq:
                    return
                PT_a, k2n_a, zbc_ab, pa = pending_dq.pop()
                dq_ps = [
                    ps_d.tile([128, S], F32, tag="ps_d", name=f"dq_ps{pa}_{a}")
                    for a in range(2)
                ]
                for i in range(NQ):
                    for a in range(2):
                        for ch in range(NCH):
                            nc.tensor.matmul(
                                dq_ps[a][a * 64 : (a + 1) * 64, ch * 512 : (ch + 1) * 512],
                                lhsT=k2n_a[:, i * 128 + a * 64 : i * 128 + (a + 1) * 64],
                                rhs=PT_a[:, (a * NQ + i) * S + ch * 512 : (a * NQ + i) * S + ch * 512 + 512],
                                start=(i == 0),
                                stop=(i == NQ - 1),
                            )
                for a in range(2):
                    nc.vector.tensor_tensor(
                        dqt2[a * 64 : (a + 1) * 64, pa * S : (pa + 1) * S],
                        dq_ps[a][a * 64 : (a + 1) * 64, :],
                        zbc_ab[a][a * 64 : (a + 1) * 64, :],
                        MUL,
                    )

            for p in range(NPAIR):
                # ---- projections: QT2/KT2 [z2, s] ----------------------------
                qt2 = sb2.tile([128, S], F16, tag="qt2")
                kt2 = sb2.tile([128, S], F16, tag="kt2")
                for wt, dst in ((wtq, qt2), (wtk, kt2)):
                    ps = ps_sc.tile([128, S], F32, tag="ps_sc", name=f"pj{p}_{dst.tensor.name}")
                    for dt in range(ND):
                        j = p * ND + dt
                        for ch in range(NCH):
                            nc.tensor.matmul(
                                ps[:, ch * 512 : (ch + 1) * 512],
                                lhsT=wt[:, j * 128 : (j + 1) * 128],
                                rhs=gt[:, dt * S + ch * 512 : dt * S + ch * 512 + 512],
                                start=(dt == 0),
                                stop=(dt == ND - 1),
                            )
                    nc.vector.tensor_copy(dst[:], ps[:])

                # ---- natural-layout transposes: Qraw / K2n [s, z2] -----------
                qraw = sb2.tile([128, S], F16, tag="qraw")
                k2n = sb2.tile([128, S], F16, tag="k2n")
                for src, dst in ((qt2, qraw), (kt2, k2n)):
                    ps = ps_sc.tile([128, S], F16, tag="ps_sc", name=f"tr{p}_{dst.tensor.name}")
                    for i in range(NQ):
                        nc.tensor.transpose(
                            ps[:, i * 128 : (i + 1) * 128],
                            src[:, i * 128 : (i + 1) * 128],
                            ident_h[:],
                        )
                    nc.vector.tensor_copy(dst[:], ps[:])

                # previous pair's deferred dQ^T burst: emitted after this pair's
                # proj/transposes so the new scores/exps win scheduler priority
                emit_dq_burst()

                # ---- fused scores/exp/accumulate loop ------------------------
                # Every op here is as WIDE as the hardware allows: this
                # runtime charges ~1.4us per engine instruction, so
                # instruction count -- not engine cycles -- is the metric.
                zsum2 = sb2.tile([128, 16], F32, tag="zsum2")  # [(q), (head, qb)]
                dk_ps = [
                    ps_d.tile([128, S], F32, tag="ps_d", name=f"dk_ps{p}_{a}")
                    for a in range(2)
                ]
                PT_all = pp.tile([128, 2 * NQ * S], F16, tag="PT", bufs=1, name=f"PT{p}")
                for i in range(NQ):
                    # scores blocks [q_i, k] for both heads (row-tiled pairs)
                    pt_s = []
                    for a in range(2):
                        ps = ps_sc.tile([128, S], F32, tag="ps_sc", name=f"sc{p}_{i}_{a}")
                        for ch in range(NCH):
                            nc.tensor.matmul(
                                ps[:, ch * 512 : (ch + 1) * 512],
                                lhsT=qt2[a * 64 : (a + 1) * 64, i * 128 : (i + 1) * 128],
                                rhs=kt2[a * 64 : (a + 1) * 64, ch * 512 : (ch + 1) * 512],
                                start=True,
                                stop=True,
                            )
                        pt_s.append(ps)
                    # P blocks + Z row-sums (one wide exp per head)
                    P_t = []
                    for a in range(2):
                        pb = pp.tile([128, S], F16, tag="P", name=f"P{p}_{i}_{a}")
                        nc.scalar.activation(
                            pb[:],
                            pt_s[a][:],
                            EXP,
                            scale=float(BETA),
                            accum_out=zsum2[:, a * NQ + i : a * NQ + i + 1],
                        )
                        P_t.append(pb)
                    # Qn block = Qraw_i / Z_i
                    q2n_t = sb4.tile([128, 128], F16, tag="q2n", name=f"q2n{p}_{i}")
                    for a in range(2):
                        zq = sb4.tile([128, 1], F32, tag="zq", name=f"zq{p}_{i}_{a}")
                        nc.vector.reciprocal(zq[:], zsum2[:, a * NQ + i : a * NQ + i + 1])
                        nc.vector.tensor_scalar_mul(
                            q2n_t[:, a * 64 : (a + 1) * 64],
                            qraw[:, i * 128 + a * 64 : i * 128 + (a + 1) * 64],
                            zq[:],
                        )
                    # dK^T += Qn_i^T P_i (col-tiled pair; each head's
                    # accumulator owns its own psum tile/banks)
                    for a in range(2):
                        for ch in range(NCH):
                            nc.tensor.matmul(
                                dk_ps[a][a * 64 : (a + 1) * 64, ch * 512 : (ch + 1) * 512],
                                lhsT=q2n_t[:, a * 64 : (a + 1) * 64],
                                rhs=P_t[a][:, ch * 512 : (ch + 1) * 512],
                                start=(i == 0),
                                stop=(i == NQ - 1),
                            )
                    # scoresT blocks [k_i, q] and PT (unnormalized exp; the
                    # dQ burst is rescaled by 1/Z afterwards via zbc)
                    for a in range(2):
                        ps = ps_sc.tile([128, S], F32, tag="ps_sc", name=f"st{p}_{i}_{a}")
                        for ch in range(NCH):
                            nc.tensor.matmul(
                                ps[:, ch * 512 : (ch + 1) * 512],
                                lhsT=kt2[a * 64 : (a + 1) * 64, i * 128 : (i + 1) * 128],
                                rhs=qt2[a * 64 : (a + 1) * 64, ch * 512 : (ch + 1) * 512],
                                start=True,
                                stop=True,
                            )
                        j = a * NQ + i
                        nc.scalar.activation(
                            PT_all[:, j * S : (j + 1) * S], ps[:], EXP, scale=float(BETA)
                        )

                # ---- Z^-1 broadcast [z2, q] then evacuate accumulators -------
                zinv2 = sb2.tile([128, 16], F32, tag="zinv2")
                nc.vector.reciprocal(zinv2[:], zsum2[:])
                zt_ps = ps_sc.tile([128, 128], F32, tag="ps_sc", name=f"ztp{p}")
                nc.tensor.transpose(zt_ps[0:16, 0:128], zinv2[:], ident[:])
                nc.vector.tensor_copy(ztsb[:], zt_ps[0:16, 0:128])
                nc.sync.dma_start(
                    zrowA[:].rearrange("p (b c) -> p b c", b=NQ), ztsb[0:NQ, :]
                )
                nc.sync.dma_start(
                    zrowB[:].rearrange("p (b c) -> p b c", b=NQ), ztsb[NQ : 2 * NQ, :]
                )
                # partition_broadcast is only correct to base partition 0 ->
                # broadcast each head's Z row across a full tile, read halves.
                zbcA = sb2.tile([128, S], F32, tag="zbcA")
                zbcB = sb2.tile([128, S], F32, tag="zbcB")
                nc.gpsimd.partition_broadcast(zbcA[:], zrowA[:])
                nc.gpsimd.partition_broadcast(zbcB[:], zrowB[:])

                for a in range(2):
                    nc.vector.tensor_copy(
                        dkt2[a * 64 : (a + 1) * 64, p * S : (p + 1) * S],
                        dk_ps[a][a * 64 : (a + 1) * 64, :],
                    )

                # (the dQ^T burst for this pair is emitted lazily -- see
                # emit_dq_burst -- so the next pair's scores/exps get priority)
                pending_dq.append((PT_all, k2n, (zbcA, zbcB), p))

            emit_dq_burst()

            # ---- output projection  sum_h dQ wq + dK wk, then RS -------------
            # Single fp16 ReduceScatter of the full [S, D] partial: rank r of
            # each 4-core batch group receives rows 256r..256r+255 -- exactly
            # the gradient rows of its OWN uploaded x chunk, so the host
            # unshard is a pure reshape.
            if comm:
                rs_in = dram.tile([S, D], F16, tag="rs_in", name=f"rsi{_rep}")
                rs_out = dram.tile([CHUNK, D], F16, tag="rs_out", name=f"rso{_rep}")
            go_all = sb1.tile([128, NQ * S], F16, tag="go_all")
            for sb in range(NQ):
                ps = ps_sc.tile([128, S], F32, tag="ps_sc", name=f"op{sb}")
                n = 0
                for p in range(NPAIR):
                    for dmat, wbase in ((dqt2, 0), (dkt2, NPAIR * D)):
                        for ch in range(NCH):
                            nc.tensor.matmul(
                                ps[:, ch * 512 : (ch + 1) * 512],
                                lhsT=dmat[:, p * S + sb * 128 : p * S + (sb + 1) * 128],
                                rhs=wn_all[:, wbase + p * D + ch * 512 : wbase + p * D + ch * 512 + 512],
                                start=(n == 0),
                                stop=(n == 2 * NPAIR - 1),
                            )
                        n += 1
                nc.vector.tensor_copy(go_all[:, sb * S : (sb + 1) * S], ps[:])
            nc.sync.dma_start(
                (rs_in if comm else gout)[:].rearrange("(b p) d -> p b d", p=128),
                go_all[:].rearrange("p (b d) -> p b d", b=NQ),
            )
            if comm:
                nc.gpsimd.collective_compute(
                    "ReduceScatter", ADD, replica_groups=GROUPS,
                    ins=[rs_in[:]], outs=[rs_out[:]],
                )
                # fused out = pos - acc on the RS'd own-chunk rows
                for m in range(2):
                    rsb = sb4.tile([128, D], F16, tag="rsb", name=f"rsb{m}")
                    nc.sync.dma_start(rsb[:], rs_out[m * 128 : (m + 1) * 128, :])
                    ob = sb4.tile([128, D], F16, tag="ob", name=f"ob{m}")
                    psb = sb2.tile([128, 1], F32, tag="psb", name=f"psb{m}")
                    nc.sync.dma_start(psb[:], posc[:, m : m + 1])
                    nc.vector.tensor_scalar(
                        ob[:], rsb[:], -1.0, psb[:], MUL, ADD
                    )
                    nc.sync.dma_start(gout[m * 128 : (m + 1) * 128, :], ob[:])

    nc.compile()
    return nc


def core_inputs(x, wq, wk, core, comm=COMM):
    """Per-core input arrays (host-side shard/layout prep, all cheap)."""
    b, r = core // 4, core % 4
    if comm:
        xck = x[b, r * CHUNK : (r + 1) * CHUNK, :].astype(np.float16)
    else:
        xck = x[b].astype(np.float16)
    h0 = 4 * (core % 4)
    wn = np.concatenate(
        [wq[h0 : h0 + 4].reshape(NPAIR * 128, D),
         wk[h0 : h0 + 4].reshape(NPAIR * 128, D)]
    ).astype(np.float16)
    if not comm:
        return {"xc": xck, "wn": wn}
    pos = np.linspace(-0.5, 0.5, S, dtype=np.float32) * np.float32(POS_SCALE)
    posc = np.ascontiguousarray(
        pos[r * CHUNK : (r + 1) * CHUNK].reshape(2, 128).T
    )
    return {"xc": xck, "wn": wn, "posc": posc}


def combine(gouts, comm=COMM):
    """Host unshard: comm mode is a pure reshape (each core returned its own
    chunk's final rows); no-comm mode sums the 4 positive partials per batch
    and applies the positional term."""
    if comm:
        return np.asarray(gouts, np.float32).reshape(B, S, D)
    pos = np.linspace(-0.5, 0.5, S, dtype=np.float32)[:, None] * np.float32(POS_SCALE)
    out = np.empty((B, S, D), np.float32)
    for b in range(B):
        acc = np.asarray(gouts[4 * b], np.float32)
        for c in range(4 * b + 1, 4 * b + 4):
            acc += np.asarray(gouts[c], np.float32)
        out[b] = pos - acc
    return out


def _build_persistent(nc):
    """One-time jitted sharded callable over the Bass NEFF (no per-call
    retracing; outputs are fully written by the kernel so no donation)."""
    import jax
    import numpy as _np
    from jax.experimental.shard_map import shard_map
    from jax.sharding import Mesh, NamedSharding, PartitionSpec

    import concourse.mybir as mybir
    from concourse import bass2jax

    bass2jax.install_neuronx_cc_hook()
    partition_name = nc.partition_id_tensor.name if nc.partition_id_tensor else None
    in_names, out_names, out_avals = [], [], []
    for alloc in nc.m.functions[0].allocations:
        if not isinstance(alloc, mybir.MemoryLocationSet):
            continue
        name = alloc.memorylocations[0].name
        if alloc.kind == "ExternalInput":
            if name != partition_name:
                in_names.append(name)
        elif alloc.kind == "ExternalOutput":
            out_names.append(name)
            out_avals.append(
                jax.core.ShapedArray(tuple(alloc.tensor_shape), mybir.dt.np(alloc.dtype))
            )
    n_params = len(in_names)
    all_in_names = list(in_names) + out_names
    if partition_name is not None:
        all_in_names.append(partition_name)

    def _body(*args):
        operands = list(args)
        if partition_name is not None:
            operands.append(bass2jax.partition_id_tensor())
        return tuple(
            bass2jax._bass_exec_p.bind(
                *operands,
                out_avals=tuple(out_avals),
                in_names=tuple(all_in_names),
                out_names=tuple(out_names),
                lowering_input_output_aliases=(),
                sim_require_finite=True,
                sim_require_nnan=True,
                nc=nc,
            )
        )

    devices = jax.devices()[:N_CORES]
    mesh = Mesh(_np.asarray(devices), ("core",))
    spec = PartitionSpec("core")
    sharded = jax.jit(
        shard_map(
            _body,
            mesh=mesh,
            in_specs=(spec,) * (n_params + len(out_names)),
            out_specs=(spec,) * len(out_names),
            check_rep=False,
        ),
        keep_unused=True,
    )
    sh = NamedSharding(mesh, spec)
    zeros = [
        jax.device_put(
            _np.zeros((N_CORES * a.shape[0],) + a.shape[1:], a.dtype), sh
        )
        for a in out_avals
    ]
    return {
        "sharded": sharded,
        "in_names": in_names,
        "out_names": out_names,
        "out_avals": out_avals,
        "sh": sh,
        "zeros": zeros,
        "jax": jax,
    }


def kernel(x, wq, wk):
    x = np.asarray(x, np.float32)
    wq = np.asarray(wq, np.float32)
    wk = np.asarray(wk, np.float32)
    if "nc" not in _CACHE:
        _CACHE["nc"] = build_nc()
    nc = _CACHE["nc"]
    if "pc" not in _CACHE:
        _CACHE["pc"] = _build_persistent(nc)
    pc = _CACHE["pc"]
    jax = pc["jax"]

    in_maps = [core_inputs(x, wq, wk, c) for c in range(N_CORES)]
    concat_in = [
        jax.device_put(
            np.concatenate([np.asarray(m[nm]) for m in in_maps], axis=0), pc["sh"]
        )
        for nm in pc["in_names"]
    ]
    outs = pc["sharded"](*concat_in, *pc["zeros"])
    g = np.asarray(outs[pc["out_names"].index("gout")])
    return combine(g.reshape(N_CORES, CHUNK if COMM else S, D))



# revision 6
# speedup vs baseline: 17.3234x; 17.3234x over previous
"""Trainium2 Bass kernel: gradient of the EnergyAttention scalar energy.

reference:
    q = einsum('bqd,hzd->bqhz', g, wq); k = einsum('bkd,hzd->bkhz', g, wk)
    scores = einsum('bqhz,bkhz->bhqk', q, k)
    E = -(logsumexp(BETA*scores, -1)/BETA).sum() + POS_SCALE*(g*pos).sum()
    out = dE/dg

Math: with E = exp(BETA*scores), Z = E.1, per (b,h):
    out[b] = -sum_h [ diag(1/Z) E K wq_h + E^T diag(1/Z) Q wk_h ] + POS_SCALE*pos

Sharding: 8 cores; core c handles batch b=c//4 and heads 4*(c%4)..4*(c%4)+3
(two head-pairs packed into the 128-partition dim).  Each core returns its
full [S, D] positive partial in fp16; the host sums 4 partials per batch and
applies the positional term (no on-device collectives -- they are slow under
this runtime).

v2 design, engineered against on-device NTFF profiles (baseline 182 us):
  * x and wq/wk are uploaded PRE-TRANSPOSED from the host (same byte count),
    so the kernel does no PE transposes for x^T / W^T.
  * Qraw/K2n [s, z2] come from XBAR DMA transposes (idle DMA engines).
  * scores blocks are contraction-64 matmuls issued strictly alternating
    head a/b so consecutive instructions land in disjoint PE row groups
    (tile_position (0,0)/(64,0)) and execute CONCURRENTLY (measured ~2x).
  * ONE exp pass per scores block (ACT) with fused row-sum accumulation for
    Z; the transposed E^T tiles for the dQ path come from XBAR DMA
    transposes of the fp16 E tiles, NOT from a scoresT matmul recompute +
    second exp pass (which would double ACT, the #2 engine).
  * dK^T accumulates per-iteration right behind each exp, col-group paired
    (tile_position (0,0)/(0,64), measured ~2x), keeping PE busy while ACT
    streams the next exp.
  * The deferred dQ^T for pair p runs as a paired burst at the start of
    pair p+1 (or the tail), and the output projection runs in two passes:
    pair-0 terms pipeline into the pair-1 loop, pair-1 terms in the tail.
"""

import numpy as np

B = 2
S = 1024
D = 1024
NH = 16
Z = 64
BETA = 1.0 / np.sqrt(np.float32(Z))
POS_SCALE = 0.001
N_CORES = 8
HPC = 4           # heads per core
NPAIR = 2         # head pairs per core
ND = D // 128     # 8 d-tiles
NQ = S // 128     # 8 q/k blocks
NCH = S // 512    # 2 moving-dim chunks

_CACHE = {}


def build_nc(reps=1):
    """Build the (SPMD, identical-per-core) Bass program.

    reps>1 repeats the whole computation (idempotent) inside one NEFF --
    used for marginal-cost timing."""
    from contextlib import ExitStack

    import concourse.mybir as mybir
    import concourse.tile as tile
    from concourse import bacc
    from concourse.masks import make_identity

    F32 = mybir.dt.float32
    F16 = mybir.dt.float16
    MUL = mybir.AluOpType.mult
    ADD = mybir.AluOpType.add
    EXP = mybir.ActivationFunctionType.Exp
    CPY = mybir.ActivationFunctionType.Copy

    nc = bacc.Bacc(
        "TRN2",
        target_bir_lowering=False,
        debug=False,
        enable_asserts=False,
        num_devices=N_CORES,
    )

    # Pre-transposed x:  xt[d, s] = x[b][s, d]
    xt_in = nc.dram_tensor("xt", [D, S], F16, kind="ExternalInput").ap()
    # Natural weights [(qk, pair, z2), d] for the output projection
    wn_in = nc.dram_tensor("wn", [2 * NPAIR * 128, D], F16, kind="ExternalInput").ap()
    # Pre-transposed weights [d % 128, (qk, pair, dt, z2)] for the projections
    wt_in = nc.dram_tensor("wt", [128, 2 * NPAIR * ND * 128], F16,
                           kind="ExternalInput").ap()
    gout = nc.dram_tensor("gout", [S, D], F16, kind="ExternalOutput").ap()

    with tile.TileContext(nc) as tc, ExitStack() as ctx:
        sb1 = ctx.enter_context(tc.tile_pool(name="sb1", bufs=1))
        sb2 = ctx.enter_context(tc.tile_pool(name="sb2", bufs=2))
        sb4 = ctx.enter_context(tc.tile_pool(name="sb4", bufs=4))
        pp = ctx.enter_context(tc.tile_pool(name="pp", bufs=1))
        # PSUM (8 banks of [128, 2KB]): "sc" rotates scores/proj/out tiles
        # ([128,1024]f32 = 2 banks, bufs=2 -> 4 banks); "d" holds the dK/dQ
        # accumulators ([128,1024]f32, bufs=2 -> 4 banks).
        ps_sc = ctx.enter_context(tc.tile_pool(name="ps_sc", bufs=2, space="PSUM"))
        ps_d = ctx.enter_context(tc.tile_pool(name="ps_d", bufs=2, space="PSUM"))

        ident = sb1.tile([128, 128], F32, tag="ident")
        make_identity(nc, ident[:])

        for _rep in range(reps):
            # ---- loads (3 merged DMAs) -----------------------------------
            gt = sb1.tile([128, ND * S], F16, tag="gt")   # [d%128, (dt, s)]
            nc.sync.dma_start(
                gt[:].rearrange("p (dt s) -> p dt s", dt=ND),
                xt_in[:].rearrange("(dt p) s -> p dt s", p=128),
            )
            wn_all = sb1.tile([128, 2 * NPAIR * D], F16, tag="wn_all")
            nc.sync.dma_start(
                wn_all[:].rearrange("p (b d) -> p b d", b=2 * NPAIR),
                wn_in[:].rearrange("(b p) d -> p b d", p=128),
            )
            wt_all = sb1.tile([128, 2 * NPAIR * ND * 128], F16, tag="wt_all")
            nc.scalar.dma_start(wt_all[:], wt_in[:])

            def wt_blk(qk, p, dt):
                j = (qk * NPAIR + p) * ND + dt
                return wt_all[:, j * 128 : (j + 1) * 128]

            # persistent across pairs
            dqt2 = sb1.tile([128, NPAIR * S], F16, tag="dqt2")  # [z2, (pair, q)]
            dkt2 = sb1.tile([128, NPAIR * S], F16, tag="dkt2")  # [z2, (pair, k)]
            go_all = sb1.tile([128, NQ * S], F16, tag="go_all")
            ztsb = sb1.tile([16, 128], F32, tag="ztsb")
            zrows = [sb1.tile([1, S], F32, tag=f"zrow{i}", name=f"zrow{i}_{_rep}")
                     for i in range(2 * NPAIR)]

            def emit_dq_burst(st, tag):
                """Deferred dQ^T(unnorm) for pair st['pa']: 8 col-group-paired
                accumulation steps over k-blocks, then Z-rescale + evac."""
                dq_ps = ps_d.tile([128, S], F32, tag="ps_d", name=f"dqp{_rep}_{tag}")
                k2n_p, PT_rp, pa = st["k2n"], st["PT_r"], st["pa"]
                for i in range(NQ):
                    for ch in range(NCH):
                        for a in range(2):
                            nc.tensor.matmul(
                                dq_ps[a * 64 : (a + 1) * 64,
                                      ch * 512 : (ch + 1) * 512],
                                lhsT=k2n_p[:, i * 128 + a * 64 : i * 128 + (a + 1) * 64],
                                rhs=PT_rp[:, a, i, ch * 512 : (ch + 1) * 512],
                                start=(i == 0),
                                stop=(i == NQ - 1),
                                tile_position=(0, a * 64),
                                skip_group_check=True,
                            )
                # Z-rescale: broadcast each head's 1/Z row and multiply
                for a in range(2):
                    zbc = sb2.tile([128, S], F32, tag="zbc", name=f"zbc{_rep}_{tag}_{a}")
                    nc.gpsimd.partition_broadcast(zbc[:], zrows[pa * 2 + a][:])
                    nc.vector.tensor_tensor(
                        dqt2[a * 64 : (a + 1) * 64, pa * S : (pa + 1) * S],
                        dq_ps[a * 64 : (a + 1) * 64, :],
                        zbc[a * 64 : (a + 1) * 64, :],
                        MUL,
                    )

            def out_chain(sb, terms, acc):
                """One q-block of the output projection: sum_t dmat_t wn_t.
                acc=False writes go_all, acc=True adds into go_all."""
                ps = ps_sc.tile([128, S], F32, tag="ps_sc", name=f"op{_rep}_{sb}_{acc}")
                for ch in range(NCH):
                    for ti, (dmat, qk, pa) in enumerate(terms):
                        nc.tensor.matmul(
                            ps[:, ch * 512 : (ch + 1) * 512],
                            lhsT=dmat[:, pa * S + sb * 128 : pa * S + (sb + 1) * 128],
                            rhs=wn_all[:, (qk * NPAIR + pa) * D + ch * 512 :
                                       (qk * NPAIR + pa) * D + ch * 512 + 512],
                            start=(ti == 0),
                            stop=(ti == len(terms) - 1),
                        )
                if acc:
                    nc.vector.tensor_tensor(
                        go_all[:, sb * S : (sb + 1) * S],
                        go_all[:, sb * S : (sb + 1) * S], ps[:], ADD)
                else:
                    nc.vector.tensor_copy(go_all[:, sb * S : (sb + 1) * S], ps[:])

            state = None

            for p in range(NPAIR):
                # ---- projections: QT2/KT2 [z2, s] ------------------------
                qt2 = sb2.tile([128, S], F16, tag="qt2")
                kt2 = sb2.tile([128, S], F16, tag="kt2")
                for qk, dst in ((0, qt2), (1, kt2)):
                    ps = ps_sc.tile([128, S], F32, tag="ps_sc",
                                    name=f"pj{_rep}_{p}_{qk}")
                    for dt in range(ND):
                        for ch in range(NCH):
                            nc.tensor.matmul(
                                ps[:, ch * 512 : (ch + 1) * 512],
                                lhsT=wt_blk(qk, p, dt),
                                rhs=gt[:, dt * S + ch * 512 : dt * S + ch * 512 + 512],
                                start=(dt == 0),
                                stop=(dt == ND - 1),
                            )
                    # split evacuations between DVE and ACT
                    if qk == 0:
                        nc.vector.tensor_copy(dst[:], ps[:])
                    else:
                        nc.scalar.activation(dst[:], ps[:], CPY)

                # ---- natural-layout Qraw / K2n [s%128, (sblk, z2)] -------
                qraw = sb2.tile([128, NQ * 128], F16, tag="qraw")
                k2n = sb2.tile([128, NQ * 128], F16, tag="k2n")
                nc.sync.dma_start_transpose(
                    qraw[:].rearrange("p (j z) -> p j z", j=NQ), qt2[:])
                nc.sync.dma_start_transpose(
                    k2n[:].rearrange("p (j z) -> p j z", j=NQ), kt2[:])

                # ---- previous pair's deferred dQ burst -------------------
                if state is not None:
                    emit_dq_burst(state, f"b{p}")
                    state = None

                # ---- fused scores/exp/dK loop ----------------------------
                zsum2 = sb2.tile([128, 16], F32, tag="zsum2")  # [(q), (head, qb)]
                dk_ps = ps_d.tile([128, S], F32, tag="ps_d", name=f"dk{_rep}_{p}")
                P_all = pp.tile([128, 2 * NQ * S], F16, tag="P", name=f"P{_rep}_{p}")
                PT_all = pp.tile([128, 2 * NQ * S], F16, tag="PT", name=f"PT{_rep}_{p}")
                PT_r = PT_all[:].rearrange("p (a j s) -> p a j s", a=2, j=NQ)

                for i in range(NQ):
                    # scores blocks [q_i, k], strictly row-group alternating
                    pt_s = [ps_sc.tile([128, S], F32, tag="ps_sc",
                                       name=f"sc{_rep}_{p}_{i}_{a}")
                            for a in range(2)]
                    for ch in range(NCH):
                        for a in range(2):
                            nc.tensor.matmul(
                                pt_s[a][:, ch * 512 : (ch + 1) * 512],
                                lhsT=qt2[a * 64 : (a + 1) * 64,
                                         i * 128 : (i + 1) * 128],
                                rhs=kt2[a * 64 : (a + 1) * 64,
                                        ch * 512 : (ch + 1) * 512],
                                start=True,
                                stop=True,
                                tile_position=(a * 64, 0),
                            )
                    # one wide exp per head (fp16 out, fused Z row-sum),
                    # then the E^T tile via XBAR DMA transpose
                    for a in range(2):
                        nc.scalar.activation(
                            P_all[:, (a * NQ + i) * S : (a * NQ + i + 1) * S],
                            pt_s[a][:],
                            EXP,
                            scale=float(BETA),
                            accum_out=zsum2[:, a * NQ + i : a * NQ + i + 1],
                        )
                        nc.sync.dma_start_transpose(
                            PT_r[:, a, :, i * 128 : (i + 1) * 128],
                            P_all[:, (a * NQ + i) * S : (a * NQ + i + 1) * S],
                        )
                    # Qn block = Qraw_i / Z_i
                    q2n_t = sb4.tile([128, 128], F16, tag="q2n",
                                     name=f"q2n{_rep}_{p}_{i}")
                    for a in range(2):
                        zq = sb4.tile([128, 1], F32, tag="zq",
                                      name=f"zq{_rep}_{p}_{i}_{a}")
                        nc.vector.reciprocal(
                            zq[:], zsum2[:, a * NQ + i : a * NQ + i + 1])
                        nc.vector.tensor_scalar_mul(
                            q2n_t[:, a * 64 : (a + 1) * 64],
                            qraw[:, i * 128 + a * 64 : i * 128 + (a + 1) * 64],
                            zq[:],
                        )
                    # dK^T += Qn_i^T E_i (col-group paired head a/b)
                    for ch in range(NCH):
                        for a in range(2):
                            nc.tensor.matmul(
                                dk_ps[a * 64 : (a + 1) * 64,
                                      ch * 512 : (ch + 1) * 512],
                                lhsT=q2n_t[:, a * 64 : (a + 1) * 64],
                                rhs=P_all[:, (a * NQ + i) * S + ch * 512 :
                                          (a * NQ + i) * S + ch * 512 + 512],
                                start=(i == 0),
                                stop=(i == NQ - 1),
                                tile_position=(0, a * 64),
                                skip_group_check=True,
                            )
                    # pair-0 output-projection terms pipeline into pair 1
                    if p == 1:
                        out_chain(i, [(dqt2, 0, 0), (dkt2, 1, 0)], acc=False)

                # ---- 1/Z rows [1, S] for the dQ rescale ------------------
                zinv2 = sb2.tile([128, 16], F32, tag="zinv2")
                nc.vector.reciprocal(zinv2[:], zsum2[:])
                zt_ps = ps_sc.tile([128, 128], F32, tag="ps_sc",
                                   name=f"ztp{_rep}_{p}")
                nc.tensor.transpose(zt_ps[0:16, 0:128], zinv2[:], ident[:])
                nc.vector.tensor_copy(ztsb[:], zt_ps[0:16, 0:128])
                for a in range(2):
                    nc.sync.dma_start(
                        zrows[p * 2 + a][:].rearrange("p (b c) -> p b c", b=NQ),
                        ztsb[a * NQ : (a + 1) * NQ, :],
                    )

                # dK evacuation (one fp32->fp16 cast covers both heads)
                nc.vector.tensor_copy(dkt2[:, p * S : (p + 1) * S], dk_ps[:])

                state = {"k2n": k2n, "PT_r": PT_r, "pa": p}

            # ---- tail: pair-1 dQ + pair-1 output-projection terms --------
            emit_dq_burst(state, "tail")
            for sb in range(NQ):
                out_chain(sb, [(dkt2, 1, 1), (dqt2, 0, 1)], acc=True)

            nc.sync.dma_start(
                gout[:].rearrange("(b p) d -> p b d", p=128),
                go_all[:].rearrange("p (b d) -> p b d", b=NQ),
            )

    nc.compile()
    return nc


def core_inputs(x, wq, wk, core):
    """Per-core input arrays (host-side shard/layout prep, all cheap)."""
    b = core // 4
    h0 = 4 * (core % 4)
    xt = np.ascontiguousarray(x[b].T).astype(np.float16)
    wq4 = wq[h0 : h0 + 4].reshape(NPAIR, 128, D)
    wk4 = wk[h0 : h0 + 4].reshape(NPAIR, 128, D)
    wn = np.concatenate(
        [wq4.reshape(NPAIR * 128, D), wk4.reshape(NPAIR * 128, D)]
    ).astype(np.float16)
    # wt[p, (qk, pair, dt, z2)] = w[qk][pair, z2, dt*128 + p]
    wstack = np.stack([wq4, wk4])                    # [qk, pair, z2, d]
    wt = (
        wstack.reshape(2, NPAIR, 128, ND, 128)       # [qk, pair, z2, dt, p]
        .transpose(4, 0, 1, 3, 2)                    # [p, qk, pair, dt, z2]
        .reshape(128, 2 * NPAIR * ND * 128)
    )
    wt = np.ascontiguousarray(wt).astype(np.float16)
    return {"xt": xt, "wn": wn, "wt": wt}


def combine(gouts):
    """Host unshard: sum the 4 positive partials per batch, apply pos term."""
    pos = np.linspace(-0.5, 0.5, S, dtype=np.float32)[:, None] * np.float32(POS_SCALE)
    out = np.empty((B, S, D), np.float32)
    for b in range(B):
        acc = np.asarray(gouts[4 * b], np.float32)
        for c in range(4 * b + 1, 4 * b + 4):
            acc += np.asarray(gouts[c], np.float32)
        out[b] = pos - acc
    return out


def _build_persistent(nc):
    """One-time jitted sharded callable over the Bass NEFF (no per-call
    retracing; outputs are fully written by the kernel so no donation)."""
    import jax
    import numpy as _np
    from jax.experimental.shard_map import shard_map
    from jax.sharding import Mesh, NamedSharding, PartitionSpec

    import concourse.mybir as mybir
    from concourse import bass2jax

    bass2jax.install_neuronx_cc_hook()
    partition_name = nc.partition_id_tensor.name if nc.partition_id_tensor else None
    in_names, out_names, out_avals = [], [], []
    for alloc in nc.m.functions[0].allocations:
        if not isinstance(alloc, mybir.MemoryLocationSet):
            continue
        name = alloc.memorylocations[0].name
        if alloc.kind == "ExternalInput":
            if name != partition_name:
                in_names.append(name)
        elif alloc.kind == "ExternalOutput":
            out_names.append(name)
            out_avals.append(
                jax.core.ShapedArray(tuple(alloc.tensor_shape), mybir.dt.np(alloc.dtype))
            )
    n_params = len(in_names)
    all_in_names = list(in_names) + out_names
    if partition_name is not None:
        all_in_names.append(partition_name)

    def _body(*args):
        operands = list(args)
        if partition_name is not None:
            operands.append(bass2jax.partition_id_tensor())
        return tuple(
            bass2jax._bass_exec_p.bind(
                *operands,
                out_avals=tuple(out_avals),
                in_names=tuple(all_in_names),
                out_names=tuple(out_names),
                lowering_input_output_aliases=(),
                sim_require_finite=True,
                sim_require_nnan=True,
                nc=nc,
            )
        )

    devices = jax.devices()[:N_CORES]
    mesh = Mesh(_np.asarray(devices), ("core",))
    spec = PartitionSpec("core")
    sharded = jax.jit(
        shard_map(
            _body,
            mesh=mesh,
            in_specs=(spec,) * (n_params + len(out_names)),
            out_specs=(spec,) * len(out_names),
            check_rep=False,
        ),
        keep_unused=True,
    )
    sh = NamedSharding(mesh, spec)
    zeros = [
        jax.device_put(
            _np.zeros((N_CORES * a.shape[0],) + a.shape[1:], a.dtype), sh
        )
        for a in out_avals
    ]
    return {
        "sharded": sharded,
        "in_names": in_names,
        "out_names": out_names,
        "out_avals": out_avals,
        "sh": sh,
        "zeros": zeros,
        "jax": jax,
    }


def kernel(x, wq, wk):
    x = np.asarray(x, np.float32)
    wq = np.asarray(wq, np.float32)
    wk = np.asarray(wk, np.float32)
    if "nc" not in _CACHE:
        _CACHE["nc"] = build_nc()
    nc = _CACHE["nc"]
    if "pc" not in _CACHE:
        _CACHE["pc"] = _build_persistent(nc)
    pc = _CACHE["pc"]
    jax = pc["jax"]

    in_maps = [core_inputs(x, wq, wk, c) for c in range(N_CORES)]
    concat_in = [
        jax.device_put(
            np.concatenate([np.asarray(m[nm]) for m in in_maps], axis=0), pc["sh"]
        )
        for nm in pc["in_names"]
    ]
    outs = pc["sharded"](*concat_in, *pc["zeros"])
    g = np.asarray(outs[pc["out_names"].index("gout")])
    return combine(g.reshape(N_CORES, S, D))


# revision 9
# speedup vs baseline: 19.0090x; 1.0973x over previous
"""Trainium2 Bass kernel: gradient of the EnergyAttention scalar energy.

reference:
    q = einsum('bqd,hzd->bqhz', g, wq); k = einsum('bkd,hzd->bkhz', g, wk)
    scores = einsum('bqhz,bkhz->bhqk', q, k)
    E = -(logsumexp(BETA*scores, -1)/BETA).sum() + POS_SCALE*(g*pos).sum()
    out = dE/dg

Math: with E = exp(BETA*scores), Z = E.1, per (b,h):
    out[b] = -sum_h [ diag(1/Z) E K wq_h + E^T diag(1/Z) Q wk_h ] + POS_SCALE*pos

Sharding: 8 cores; core c handles batch b=c//4 and heads 4*(c%4)..4*(c%4)+3
(two head-pairs packed into the 128-partition dim).  Each core returns its
full [S, D] positive partial in fp16; the host sums 4 partials per batch and
applies the positional term (no on-device collectives -- slow under this
runtime).

v3 design, engineered against on-device NTFF profiles (baseline 182 us,
v2 134 us):
  * x and wq/wk are uploaded PRE-TRANSPOSED from the host (same byte count)
    -- no PE transposes for x^T / W^T.  Qraw/K2n [s, z2] come from XBAR DMA
    transposes (idle DMA engines, ~1 us each).
  * scores blocks are contraction-64 matmuls issued strictly alternating
    head a/b: consecutive instructions land in disjoint PE row groups
    (tile_position (0,0)/(64,0)) and execute CONCURRENTLY (measured ~2x).
    dK/dQ are M=64 matmuls, col-group paired the same way ((0,0)/(0,64)).
  * ONE exp pass per scores block (ACT) with fused row-sum accumulation for
    Z; the transposed E^T tiles for the dQ path come from XBAR DMA
    transposes of the fp16 E tiles, NOT a scoresT recompute + second exp
    pass (which would double ACT time, the #2 engine).
  * Software pipelining keeps every engine fed and avoids PE FIFO
    head-of-line stalls: dK_i runs one iteration behind exp_i; pair-1's
    projections fill pair-0's loop; pair-0's deferred dQ burst covers the
    pair transition; pair-0's output-projection terms run inside pair-1's
    loop; the tail interleaves pair-1's dQ with the dK-half of the final
    output pass, and the gout DMA is chunked per q-block.
"""

import numpy as np

B = 2
S = 1024
D = 1024
NH = 16
Z = 64
BETA = 1.0 / np.sqrt(np.float32(Z))
POS_SCALE = 0.001
N_CORES = 8
HPC = 4           # heads per core
NPAIR = 2         # head pairs per core
ND = D // 128     # 8 d-tiles
NQ = S // 128     # 8 q/k blocks
NCH = S // 512    # 2 moving-dim chunks

_CACHE = {}


def build_nc(reps=1):
    """Build the (SPMD, identical-per-core) Bass program.

    reps>1 repeats the whole computation (idempotent) inside one NEFF --
    used for marginal-cost timing."""
    from contextlib import ExitStack

    import concourse.mybir as mybir
    import concourse.tile as tile
    from concourse import bacc
    from concourse.masks import make_identity

    F32 = mybir.dt.float32
    F16 = mybir.dt.float16
    MUL = mybir.AluOpType.mult
    ADD = mybir.AluOpType.add
    EXP = mybir.ActivationFunctionType.Exp

    nc = bacc.Bacc(
        "TRN2",
        target_bir_lowering=False,
        debug=False,
        enable_asserts=False,
        num_devices=N_CORES,
    )

    # Pre-transposed x:  xt[d, s] = x[b][s, d]
    xt_in = nc.dram_tensor("xt", [D, S], F16, kind="ExternalInput").ap()
    # Natural weights [(qk, pair, z2), d] for the output projection
    wn_in = nc.dram_tensor("wn", [2 * NPAIR * 128, D], F16, kind="ExternalInput").ap()
    # Pre-transposed weights [d % 128, (qk, pair, dt, z2)] for the projections
    wt_in = nc.dram_tensor("wt", [128, 2 * NPAIR * ND * 128], F16,
                           kind="ExternalInput").ap()
    gout = nc.dram_tensor("gout", [S, D], F16, kind="ExternalOutput").ap()

    with tile.TileContext(nc) as tc, ExitStack() as ctx:
        sb1 = ctx.enter_context(tc.tile_pool(name="sb1", bufs=1))
        sb2 = ctx.enter_context(tc.tile_pool(name="sb2", bufs=2))
        sb4 = ctx.enter_context(tc.tile_pool(name="sb4", bufs=4))
        pp = ctx.enter_context(tc.tile_pool(name="pp", bufs=1))
        # PSUM (8 banks of [128, 2KB]): "sc" rotates scores/out tiles
        # ([128,1024]f32 = 2 banks, bufs=2 -> 4 banks); "d" rotates the
        # dK/dQ accumulators and the interleaved projection tiles (4 banks).
        ps_sc = ctx.enter_context(tc.tile_pool(name="ps_sc", bufs=2, space="PSUM"))
        ps_d = ctx.enter_context(tc.tile_pool(name="ps_d", bufs=2, space="PSUM"))

        ident = sb1.tile([128, 128], F32, tag="ident")
        make_identity(nc, ident[:])

        for _rep in range(reps):
            # ---- loads (3 merged DMAs) -----------------------------------
            gt = sb1.tile([128, ND * S], F16, tag="gt")   # [d%128, (dt, s)]
            nc.sync.dma_start(
                gt[:].rearrange("p (dt s) -> p dt s", dt=ND),
                xt_in[:].rearrange("(dt p) s -> p dt s", p=128),
            )
            wn_all = sb1.tile([128, 2 * NPAIR * D], F16, tag="wn_all")
            nc.sync.dma_start(
                wn_all[:].rearrange("p (b d) -> p b d", b=2 * NPAIR),
                wn_in[:].rearrange("(b p) d -> p b d", p=128),
            )
            wt_all = sb1.tile([128, 2 * NPAIR * ND * 128], F16, tag="wt_all")
            nc.scalar.dma_start(wt_all[:], wt_in[:])

            def wt_blk(qk, p, dt):
                j = (qk * NPAIR + p) * ND + dt
                return wt_all[:, j * 128 : (j + 1) * 128]

            # persistent across pairs
            dqt2 = sb1.tile([128, NPAIR * S], F16, tag="dqt2")  # [z2, (pair, q)]
            dkt2 = sb1.tile([128, NPAIR * S], F16, tag="dkt2")  # [z2, (pair, k)]
            go_all = sb1.tile([128, NQ * S], F16, tag="go_all")
            ztsb = sb1.tile([16, 128], F32, tag="ztsb")
            zrows = [sb1.tile([1, S], F32, tag=f"zrow{i}", name=f"zrow{i}_{_rep}")
                     for i in range(2 * NPAIR)]

            pairs = []  # per-pair tiles

            def proj_chunk(p, qk, ps, dts, dst):
                """Two d-tiles of the Q/K projection for pair p."""
                for dt in dts:
                    for ch in range(NCH):
                        nc.tensor.matmul(
                            ps[:, ch * 512 : (ch + 1) * 512],
                            lhsT=wt_blk(qk, p, dt),
                            rhs=gt[:, dt * S + ch * 512 : dt * S + ch * 512 + 512],
                            start=(dt == 0),
                            stop=(dt == ND - 1),
                        )
                if dst is not None and dts[-1] == ND - 1:
                    nc.vector.tensor_copy(dst[:], ps[:])

            def scores_exp(p, i):
                """Scores blocks [q_i, k] (row-group paired) + one exp/head."""
                qt2, kt2 = pairs[p]["qt2"], pairs[p]["kt2"]
                pt_s = [ps_sc.tile([128, S], F32, tag="ps_sc",
                                   name=f"sc{_rep}_{p}_{i}_{a}")
                        for a in range(2)]
                for ch, a in ((0, 0), (0, 1), (1, 1), (1, 0)):
                    nc.tensor.matmul(
                        pt_s[a][:, ch * 512 : (ch + 1) * 512],
                        lhsT=qt2[a * 64 : (a + 1) * 64, i * 128 : (i + 1) * 128],
                        rhs=kt2[a * 64 : (a + 1) * 64, ch * 512 : (ch + 1) * 512],
                        start=True,
                        stop=True,
                        tile_position=(a * 64, 0),
                    )
                P_all, PT_r = pairs[p]["P_all"], pairs[p]["PT_r"]
                zsum2 = pairs[p]["zsum2"]
                for a in range(2):
                    nc.scalar.activation(
                        P_all[:, (a * NQ + i) * S : (a * NQ + i + 1) * S],
                        pt_s[a][:],
                        EXP,
                        scale=float(BETA),
                        accum_out=zsum2[:, a * NQ + i : a * NQ + i + 1],
                    )
                    nc.sync.dma_start_transpose(
                        PT_r[:, a, :, i * 128 : (i + 1) * 128],
                        P_all[:, (a * NQ + i) * S : (a * NQ + i + 1) * S],
                    )

            def dk_step(p, j):
                """q2n_j then dK^T += Qn_j^T E_j (col-group paired); emitted
                one iteration behind exp_j so the PE never waits on ACT."""
                qraw, zsum2 = pairs[p]["qraw"], pairs[p]["zsum2"]
                P_all, dk_ps = pairs[p]["P_all"], pairs[p]["dk_ps"]
                q2n_t = sb4.tile([128, 128], F16, tag="q2n",
                                 name=f"q2n{_rep}_{p}_{j}")
                for a in range(2):
                    zq = sb4.tile([128, 1], F32, tag="zq",
                                  name=f"zq{_rep}_{p}_{j}_{a}")
                    nc.vector.reciprocal(
                        zq[:], zsum2[:, a * NQ + j : a * NQ + j + 1])
                    nc.vector.tensor_scalar_mul(
                        q2n_t[:, a * 64 : (a + 1) * 64],
                        qraw[:, j * 128 + a * 64 : j * 128 + (a + 1) * 64],
                        zq[:],
                    )
                for ch, a in ((0, 0), (0, 1), (1, 1), (1, 0)):
                    nc.tensor.matmul(
                        dk_ps[a * 64 : (a + 1) * 64, ch * 512 : (ch + 1) * 512],
                        lhsT=q2n_t[:, a * 64 : (a + 1) * 64],
                        rhs=P_all[:, (a * NQ + j) * S + ch * 512 :
                                  (a * NQ + j) * S + ch * 512 + 512],
                        start=(j == 0),
                        stop=(j == NQ - 1),
                        tile_position=(0, a * 64),
                        skip_group_check=True,
                    )

            def emit_dq_burst(p, tag):
                """Deferred dQ^T(unnorm): 8 col-group-paired accumulation
                steps over k-blocks, then Z-rescale into dqt2."""
                dq_ps = ps_d.tile([128, S], F32, tag="ps_d", name=f"dqp{_rep}_{tag}")
                k2n_p, PT_rp = pairs[p]["k2n"], pairs[p]["PT_r"]
                for i in range(NQ):
                    for ch, a in ((0, 0), (0, 1), (1, 1), (1, 0)):
                        nc.tensor.matmul(
                            dq_ps[a * 64 : (a + 1) * 64, ch * 512 : (ch + 1) * 512],
                            lhsT=k2n_p[:, i * 128 + a * 64 : i * 128 + (a + 1) * 64],
                            rhs=PT_rp[:, a, i, ch * 512 : (ch + 1) * 512],
                            start=(i == 0),
                            stop=(i == NQ - 1),
                            tile_position=(0, a * 64),
                            skip_group_check=True,
                        )
                for a in range(2):
                    zbc = sb2.tile([128, S], F32, tag="zbc", name=f"zbc{_rep}_{tag}_{a}")
                    nc.gpsimd.partition_broadcast(zbc[:], zrows[p * 2 + a][:])
                    nc.vector.tensor_tensor(
                        dqt2[a * 64 : (a + 1) * 64, p * S : (p + 1) * S],
                        dq_ps[a * 64 : (a + 1) * 64, :],
                        zbc[a * 64 : (a + 1) * 64, :],
                        MUL,
                    )

            def out_chain(sb, terms, acc):
                """One q-block of the output projection: sum_t dmat_t wn_t."""
                ps = ps_sc.tile([128, S], F32, tag="ps_sc",
                                name=f"op{_rep}_{sb}_{acc}")
                for ch in range(NCH):
                    for ti, (dmat, qk, pa) in enumerate(terms):
                        nc.tensor.matmul(
                            ps[:, ch * 512 : (ch + 1) * 512],
                            lhsT=dmat[:, pa * S + sb * 128 : pa * S + (sb + 1) * 128],
                            rhs=wn_all[:, (qk * NPAIR + pa) * D + ch * 512 :
                                       (qk * NPAIR + pa) * D + ch * 512 + 512],
                            start=(ti == 0),
                            stop=(ti == len(terms) - 1),
                        )
                if acc:
                    nc.vector.tensor_tensor(
                        go_all[:, sb * S : (sb + 1) * S],
                        go_all[:, sb * S : (sb + 1) * S], ps[:], ADD)
                else:
                    nc.vector.tensor_copy(go_all[:, sb * S : (sb + 1) * S], ps[:])

            def pair_end(p):
                """1/Z rows [1, S] for the dQ rescale + dK evacuation."""
                zsum2, dk_ps = pairs[p]["zsum2"], pairs[p]["dk_ps"]
                zinv2 = sb2.tile([128, 16], F32, tag="zinv2")
                nc.vector.reciprocal(zinv2[:], zsum2[:])
                zt_ps = ps_sc.tile([128, 128], F32, tag="ps_sc",
                                   name=f"ztp{_rep}_{p}")
                nc.tensor.transpose(zt_ps[0:16, 0:128], zinv2[:], ident[:])
                nc.vector.tensor_copy(ztsb[:], zt_ps[0:16, 0:128])
                for a in range(2):
                    nc.scalar.dma_start(
                        zrows[p * 2 + a][:].rearrange("p (b c) -> p b c", b=NQ),
                        ztsb[a * NQ : (a + 1) * NQ, :],
                    )
                nc.vector.tensor_copy(dkt2[:, p * S : (p + 1) * S], dk_ps[:])

            def alloc_pair(p):
                d = {}
                d["qt2"] = sb2.tile([128, S], F16, tag="qt2", name=f"qt2_{_rep}_{p}")
                d["kt2"] = sb2.tile([128, S], F16, tag="kt2", name=f"kt2_{_rep}_{p}")
                d["qraw"] = sb2.tile([128, NQ * 128], F16, tag="qraw",
                                     name=f"qraw_{_rep}_{p}")
                d["k2n"] = sb2.tile([128, NQ * 128], F16, tag="k2n",
                                    name=f"k2n_{_rep}_{p}")
                d["zsum2"] = sb2.tile([128, 16], F32, tag="zsum2",
                                      name=f"zsum2_{_rep}_{p}")
                d["P_all"] = pp.tile([128, 2 * NQ * S], F16, tag="P",
                                     name=f"P{_rep}_{p}")
                d["PT_all"] = pp.tile([128, 2 * NQ * S], F16, tag="PT",
                                      name=f"PT{_rep}_{p}")
                d["PT_r"] = d["PT_all"][:].rearrange("p (a j s) -> p a j s",
                                                     a=2, j=NQ)
                return d

            def qraw_k2n(p):
                nc.sync.dma_start_transpose(
                    pairs[p]["qraw"][:].rearrange("p (j z) -> p j z", j=NQ),
                    pairs[p]["qt2"][:])
                nc.sync.dma_start_transpose(
                    pairs[p]["k2n"][:].rearrange("p (j z) -> p j z", j=NQ),
                    pairs[p]["kt2"][:])

            # ================= schedule =================
            pairs.append(alloc_pair(0))
            pairs.append(alloc_pair(1))

            # pair-0 projections up front
            for qk in range(2):
                ps = ps_d.tile([128, S], F32, tag="ps_d", name=f"pj{_rep}_0_{qk}")
                proj_chunk(0, qk, ps, list(range(ND)),
                           pairs[0]["qt2"] if qk == 0 else pairs[0]["kt2"])
            qraw_k2n(0)

            # pair-0 loop; pair-1 projections ride along (4 MMs per slot)
            pj_ps = {}
            dk0 = pairs[0]["dk_ps"] = ps_d.tile([128, S], F32, tag="ps_d",
                                                name=f"dk{_rep}_0")
            for i in range(NQ + 1):
                if i < NQ:
                    scores_exp(0, i)
                if i < 4:
                    if i == 0:
                        pj_ps[0] = ps_d.tile([128, S], F32, tag="ps_d",
                                             name=f"pj{_rep}_1_0")
                    proj_chunk(1, 0, pj_ps[0], [2 * i, 2 * i + 1],
                               pairs[1]["qt2"])
                elif i < NQ:
                    if i == 4:
                        pj_ps[1] = ps_d.tile([128, S], F32, tag="ps_d",
                                             name=f"pj{_rep}_1_1")
                    proj_chunk(1, 1, pj_ps[1], [2 * (i - 4), 2 * (i - 4) + 1],
                               pairs[1]["kt2"])
                if i >= 1:
                    dk_step(0, i - 1)
            pair_end(0)
            qraw_k2n(1)

            # pair-1 loop; pair-0's dQ burst covers the transition, pair-0's
            # output-projection terms ride along
            emit_dq_burst(0, "b1")
            pairs[1]["dk_ps"] = ps_d.tile([128, S], F32, tag="ps_d",
                                          name=f"dk{_rep}_1")
            for i in range(NQ + 1):
                if i < NQ:
                    scores_exp(1, i)
                if i >= 1:
                    dk_step(1, i - 1)
                if i < NQ:
                    out_chain(i, [(dqt2, 0, 0), (dkt2, 1, 0)], acc=False)
            pair_end(1)

            # tail: pair-1 dQ + the pair-1 output-projection terms.
            # dk-half chains first (ready at pair_end), dq-half after the
            # rescale; evacuations split DVE/GpSimd; gout DMA chunked.
            emit_dq_burst(1, "tail")
            for sb in range(NQ):
                out_chain(sb, [(dkt2, 1, 1), (dqt2, 0, 1)], acc=True)
                nc.sync.dma_start(
                    gout[sb * 128 : (sb + 1) * 128, :],
                    go_all[:, sb * S : (sb + 1) * S],
                )

    nc.compile()
    return nc


def core_inputs(x, wq, wk, core):
    """Per-core input arrays (host-side shard/layout prep, all cheap)."""
    b = core // 4
    h0 = 4 * (core % 4)
    xt = np.ascontiguousarray(x[b].T).astype(np.float16)
    wq4 = wq[h0 : h0 + 4].reshape(NPAIR, 128, D)
    wk4 = wk[h0 : h0 + 4].reshape(NPAIR, 128, D)
    wn = np.concatenate(
        [wq4.reshape(NPAIR * 128, D), wk4.reshape(NPAIR * 128, D)]
    ).astype(np.float16)
    # wt[p, (qk, pair, dt, z2)] = w[qk][pair, z2, dt*128 + p]
    wstack = np.stack([wq4, wk4])                    # [qk, pair, z2, d]
    wt = (
        wstack.reshape(2, NPAIR, 128, ND, 128)       # [qk, pair, z2, dt, p]
        .transpose(4, 0, 1, 3, 2)                    # [p, qk, pair, dt, z2]
        .reshape(128, 2 * NPAIR * ND * 128)
    )
    wt = np.ascontiguousarray(wt).astype(np.float16)
    return {"xt": xt, "wn": wn, "wt": wt}


def combine(gouts):
    """Host unshard: sum the 4 positive partials per batch, apply pos term."""
    pos = np.linspace(-0.5, 0.5, S, dtype=np.float32)[:, None] * np.float32(POS_SCALE)
    out = np.empty((B, S, D), np.float32)
    for b in range(B):
        acc = np.asarray(gouts[4 * b], np.float32)
        for c in range(4 * b + 1, 4 * b + 4):
            acc += np.asarray(gouts[c], np.float32)
        out[b] = pos - acc
    return out


def _build_persistent(nc):
    """One-time jitted sharded callable over the Bass NEFF (no per-call
    retracing; outputs are fully written by the kernel so no donation)."""
    import jax
    import numpy as _np
    from jax.experimental.shard_map import shard_map
    from jax.sharding import Mesh, NamedSharding, PartitionSpec

    import concourse.mybir as mybir
    from concourse import bass2jax

    bass2jax.install_neuronx_cc_hook()
    partition_name = nc.partition_id_tensor.name if nc.partition_id_tensor else None
    in_names, out_names, out_avals = [], [], []
    for alloc in nc.m.functions[0].allocations:
        if not isinstance(alloc, mybir.MemoryLocationSet):
            continue
        name = alloc.memorylocations[0].name
        if alloc.kind == "ExternalInput":
            if name != partition_name:
                in_names.append(name)
        elif alloc.kind == "ExternalOutput":
            out_names.append(name)
            out_avals.append(
                jax.core.ShapedArray(tuple(alloc.tensor_shape), mybir.dt.np(alloc.dtype))
            )
    n_params = len(in_names)
    all_in_names = list(in_names) + out_names
    if partition_name is not None:
        all_in_names.append(partition_name)

    def _body(*args):
        operands = list(args)
        if partition_name is not None:
            operands.append(bass2jax.partition_id_tensor())
        return tuple(
            bass2jax._bass_exec_p.bind(
                *operands,
                out_avals=tuple(out_avals),
                in_names=tuple(all_in_names),
                out_names=tuple(out_names),
                lowering_input_output_aliases=(),
                sim_require_finite=True,
                sim_require_nnan=True,
                nc=nc,
            )
        )

    devices = jax.devices()[:N_CORES]
    mesh = Mesh(_np.asarray(devices), ("core",))
    spec = PartitionSpec("core")
    sharded = jax.jit(
        shard_map(
            _body,
            mesh=mesh,
            in_specs=(spec,) * (n_params + len(out_names)),
            out_specs=(spec,) * len(out_names),
            check_rep=False,
        ),
        keep_unused=True,
    )
    sh = NamedSharding(mesh, spec)
    zeros = [
        jax.device_put(
            _np.zeros((N_CORES * a.shape[0],) + a.shape[1:], a.dtype), sh
        )
        for a in out_avals
    ]
    return {
        "sharded": sharded,
        "in_names": in_names,
        "out_names": out_names,
        "out_avals": out_avals,
        "sh": sh,
        "zeros": zeros,
        "jax": jax,
    }


def kernel(x, wq, wk):
    x = np.asarray(x, np.float32)
    wq = np.asarray(wq, np.float32)
    wk = np.asarray(wk, np.float32)
    if "nc" not in _CACHE:
        _CACHE["nc"] = build_nc()
    nc = _CACHE["nc"]
    if "pc" not in _CACHE:
        _CACHE["pc"] = _build_persistent(nc)
    pc = _CACHE["pc"]
    jax = pc["jax"]

    in_maps = [core_inputs(x, wq, wk, c) for c in range(N_CORES)]
    concat_in = [
        jax.device_put(
            np.concatenate([np.asarray(m[nm]) for m in in_maps], axis=0), pc["sh"]
        )
        for nm in pc["in_names"]
    ]
    outs = pc["sharded"](*concat_in, *pc["zeros"])
    g = np.asarray(outs[pc["out_names"].index("gout")])
    return combine(g.reshape(N_CORES, S, D))


# revision 10
# speedup vs baseline: 20.5170x; 1.0793x over previous
"""Trainium2 Bass kernel: gradient of the EnergyAttention scalar energy.

reference:
    q = einsum('bqd,hzd->bqhz', g, wq); k = einsum('bkd,hzd->bkhz', g, wk)
    scores = einsum('bqhz,bkhz->bhqk', q, k)
    E = -(logsumexp(BETA*scores, -1)/BETA).sum() + POS_SCALE*(g*pos).sum()
    out = dE/dg

Math: with E = exp(BETA*scores), Z = E.1, per (b,h):
    out[b] = -sum_h [ diag(1/Z) E K wq_h + E^T diag(1/Z) Q wk_h ] + POS_SCALE*pos

Sharding: 8 cores; core c handles batch b=c//4 and heads 4*(c%4)..4*(c%4)+3
(two head-pairs packed into the 128-partition dim).  Each core returns its
full [S, D] positive partial in fp16; the host sums 4 partials per batch and
applies the positional term (no on-device collectives -- slow under this
runtime).

v3 design, engineered against on-device NTFF profiles (baseline 182 us,
v2 134 us):
  * x and wq/wk are uploaded PRE-TRANSPOSED from the host (same byte count)
    -- no PE transposes for x^T / W^T.  Qraw/K2n [s, z2] come from XBAR DMA
    transposes (idle DMA engines, ~1 us each).
  * scores blocks are contraction-64 matmuls issued strictly alternating
    head a/b: consecutive instructions land in disjoint PE row groups
    (tile_position (0,0)/(64,0)) and execute CONCURRENTLY (measured ~2x).
    dK/dQ are M=64 matmuls, col-group paired the same way ((0,0)/(0,64)).
  * ONE exp pass per scores block (ACT) with fused row-sum accumulation for
    Z; the transposed E^T tiles for the dQ path come from XBAR DMA
    transposes of the fp16 E tiles, NOT a scoresT recompute + second exp
    pass (which would double ACT time, the #2 engine).
  * Software pipelining keeps every engine fed and avoids PE FIFO
    head-of-line stalls: dK_i runs one iteration behind exp_i; pair-1's
    projections fill pair-0's loop; pair-0's deferred dQ burst covers the
    pair transition; pair-0's output-projection terms run inside pair-1's
    loop; the tail interleaves pair-1's dQ with the dK-half of the final
    output pass, and the gout DMA is chunked per q-block.
"""

import numpy as np

B = 2
S = 1024
D = 1024
NH = 16
Z = 64
BETA = 1.0 / np.sqrt(np.float32(Z))
POS_SCALE = 0.001
N_CORES = 8
HPC = 4           # heads per core
NPAIR = 2         # head pairs per core
ND = D // 128     # 8 d-tiles
NQ = S // 128     # 8 q/k blocks
NCH = S // 512    # 2 moving-dim chunks

_CACHE = {}


def build_nc(reps=1):
    """Build the (SPMD, identical-per-core) Bass program.

    reps>1 repeats the whole computation (idempotent) inside one NEFF --
    used for marginal-cost timing."""
    from contextlib import ExitStack

    import concourse.mybir as mybir
    import concourse.tile as tile
    from concourse import bacc
    from concourse.masks import make_identity

    F32 = mybir.dt.float32
    F16 = mybir.dt.float16
    MUL = mybir.AluOpType.mult
    ADD = mybir.AluOpType.add
    EXP = mybir.ActivationFunctionType.Exp

    nc = bacc.Bacc(
        "TRN2",
        target_bir_lowering=False,
        debug=False,
        enable_asserts=False,
        num_devices=N_CORES,
    )

    # Pre-transposed x:  xt[d, s] = x[b][s, d]
    xt_in = nc.dram_tensor("xt", [D, S], F16, kind="ExternalInput").ap()
    # Natural weights [(qk, pair, z2), d] for the output projection
    wn_in = nc.dram_tensor("wn", [2 * NPAIR * 128, D], F16, kind="ExternalInput").ap()
    # Pre-transposed weights [d % 128, (qk, pair, dt, z2)] for the projections
    wt_in = nc.dram_tensor("wt", [128, 2 * NPAIR * ND * 128], F16,
                           kind="ExternalInput").ap()
    gout = nc.dram_tensor("gout", [S, D], F16, kind="ExternalOutput").ap()

    with tile.TileContext(nc) as tc, ExitStack() as ctx:
        sb1 = ctx.enter_context(tc.tile_pool(name="sb1", bufs=1))
        sb2 = ctx.enter_context(tc.tile_pool(name="sb2", bufs=2))
        sb4 = ctx.enter_context(tc.tile_pool(name="sb4", bufs=4))
        pp = ctx.enter_context(tc.tile_pool(name="pp", bufs=1))
        # PSUM (8 banks of [128, 2KB]): "sc" rotates scores/out tiles
        # ([128,1024]f32 = 2 banks, bufs=2 -> 4 banks); "d" rotates the
        # dK/dQ accumulators and the interleaved projection tiles (4 banks).
        ps_sc = ctx.enter_context(tc.tile_pool(name="ps_sc", bufs=2, space="PSUM"))
        ps_d = ctx.enter_context(tc.tile_pool(name="ps_d", bufs=2, space="PSUM"))

        ident = sb1.tile([128, 128], F32, tag="ident")
        make_identity(nc, ident[:])

        for _rep in range(reps):
            # ---- loads (3 merged DMAs) -----------------------------------
            gt = sb1.tile([128, ND * S], F16, tag="gt")   # [d%128, (dt, s)]
            gt_r = gt[:].rearrange("p (dt s) -> p dt s", dt=ND)
            xt_r = xt_in[:].rearrange("(dt p) s -> p dt s", p=128)
            half = ND // 2
            nc.sync.dma_start(gt_r[:, 0:half, :], xt_r[:, 0:half, :])
            nc.sync.dma_start(gt_r[:, half:ND, :], xt_r[:, half:ND, :])
            wn_all = sb1.tile([128, 2 * NPAIR * D], F16, tag="wn_all")
            nc.sync.dma_start(
                wn_all[:].rearrange("p (b d) -> p b d", b=2 * NPAIR),
                wn_in[:].rearrange("(b p) d -> p b d", p=128),
            )
            wt_all = sb1.tile([128, 2 * NPAIR * ND * 128], F16, tag="wt_all")
            nc.scalar.dma_start(wt_all[:], wt_in[:])

            def wt_blk(qk, p, dt):
                j = (qk * NPAIR + p) * ND + dt
                return wt_all[:, j * 128 : (j + 1) * 128]

            # persistent across pairs
            dqt2 = sb1.tile([128, NPAIR * S], F16, tag="dqt2")  # [z2, (pair, q)]
            dkt2 = sb1.tile([128, NPAIR * S], F16, tag="dkt2")  # [z2, (pair, k)]
            go_all = sb1.tile([128, NQ * S], F16, tag="go_all")
            ztsb = sb1.tile([16, 128], F32, tag="ztsb")
            zrows = [sb1.tile([1, S], F32, tag=f"zrow{i}", name=f"zrow{i}_{_rep}")
                     for i in range(2 * NPAIR)]

            pairs = []  # per-pair tiles

            def proj_chunk(p, qk, ps, dts, dst):
                """Two d-tiles of the Q/K projection for pair p."""
                for dt in dts:
                    for ch in range(NCH):
                        nc.tensor.matmul(
                            ps[:, ch * 512 : (ch + 1) * 512],
                            lhsT=wt_blk(qk, p, dt),
                            rhs=gt[:, dt * S + ch * 512 : dt * S + ch * 512 + 512],
                            start=(dt == 0),
                            stop=(dt == ND - 1),
                        )
                if dst is not None and dts[-1] == ND - 1:
                    nc.vector.tensor_copy(dst[:], ps[:])

            def scores_exp(p, i):
                """Scores blocks [q_i, k] (row-group paired) + one exp/head."""
                qt2, kt2 = pairs[p]["qt2"], pairs[p]["kt2"]
                pt_s = [ps_sc.tile([128, S], F32, tag="ps_sc",
                                   name=f"sc{_rep}_{p}_{i}_{a}")
                        for a in range(2)]
                for ch, a in ((0, 0), (0, 1), (1, 1), (1, 0)):
                    nc.tensor.matmul(
                        pt_s[a][:, ch * 512 : (ch + 1) * 512],
                        lhsT=qt2[a * 64 : (a + 1) * 64, i * 128 : (i + 1) * 128],
                        rhs=kt2[a * 64 : (a + 1) * 64, ch * 512 : (ch + 1) * 512],
                        start=True,
                        stop=True,
                        tile_position=(a * 64, 0),
                    )
                P_all, PT_r = pairs[p]["P_all"], pairs[p]["PT_r"]
                zsum2 = pairs[p]["zsum2"]
                for a in range(2):
                    nc.scalar.activation(
                        P_all[:, (a * NQ + i) * S : (a * NQ + i + 1) * S],
                        pt_s[a][:],
                        EXP,
                        scale=float(BETA),
                        accum_out=zsum2[:, a * NQ + i : a * NQ + i + 1],
                    )
                    nc.sync.dma_start_transpose(
                        PT_r[:, a, :, i * 128 : (i + 1) * 128],
                        P_all[:, (a * NQ + i) * S : (a * NQ + i + 1) * S],
                    )

            def dk_step(p, j):
                """q2n_j then dK^T += Qn_j^T E_j (col-group paired); emitted
                one iteration behind exp_j so the PE never waits on ACT."""
                qraw, zsum2 = pairs[p]["qraw"], pairs[p]["zsum2"]
                P_all, dk_ps = pairs[p]["P_all"], pairs[p]["dk_ps"]
                q2n_t = sb4.tile([128, 128], F16, tag="q2n",
                                 name=f"q2n{_rep}_{p}_{j}")
                for a in range(2):
                    zq = sb4.tile([128, 1], F32, tag="zq",
                                  name=f"zq{_rep}_{p}_{j}_{a}")
                    nc.vector.reciprocal(
                        zq[:], zsum2[:, a * NQ + j : a * NQ + j + 1])
                    nc.vector.tensor_scalar_mul(
                        q2n_t[:, a * 64 : (a + 1) * 64],
                        qraw[:, j * 128 + a * 64 : j * 128 + (a + 1) * 64],
                        zq[:],
                    )
                for ch, a in ((0, 0), (0, 1), (1, 1), (1, 0)):
                    nc.tensor.matmul(
                        dk_ps[a * 64 : (a + 1) * 64, ch * 512 : (ch + 1) * 512],
                        lhsT=q2n_t[:, a * 64 : (a + 1) * 64],
                        rhs=P_all[:, (a * NQ + j) * S + ch * 512 :
                                  (a * NQ + j) * S + ch * 512 + 512],
                        start=(j == 0),
                        stop=(j == NQ - 1),
                        tile_position=(0, a * 64),
                        skip_group_check=True,
                    )

            def emit_dq_burst(p, tag):
                """Deferred dQ^T(unnorm): 8 col-group-paired accumulation
                steps over k-blocks, then Z-rescale into dqt2."""
                dq_ps = ps_d.tile([128, S], F32, tag="ps_d", name=f"dqp{_rep}_{tag}")
                k2n_p, PT_rp = pairs[p]["k2n"], pairs[p]["PT_r"]
                for ch in range(NCH):
                    for i in range(NQ):
                        for a in ((0, 1) if i % 2 == 0 else (1, 0)):
                            nc.tensor.matmul(
                                dq_ps[a * 64 : (a + 1) * 64,
                                      ch * 512 : (ch + 1) * 512],
                                lhsT=k2n_p[:, i * 128 + a * 64 : i * 128 + (a + 1) * 64],
                                rhs=PT_rp[:, a, i, ch * 512 : (ch + 1) * 512],
                                start=(i == 0),
                                stop=(i == NQ - 1),
                                tile_position=(0, a * 64),
                                skip_group_check=True,
                            )
                for a in range(2):
                    zbc = sb2.tile([128, S], F32, tag="zbc", name=f"zbc{_rep}_{tag}_{a}")
                    nc.gpsimd.partition_broadcast(zbc[:], zrows[p * 2 + a][:])
                    nc.vector.tensor_tensor(
                        dqt2[a * 64 : (a + 1) * 64, p * S : (p + 1) * S],
                        dq_ps[a * 64 : (a + 1) * 64, :],
                        zbc[a * 64 : (a + 1) * 64, :],
                        MUL,
                    )

            def out_chain(sb, terms, acc, pool=None, ptag=None):
                """One q-block of the output projection: sum_t dmat_t wn_t."""
                pool = pool if pool is not None else ps_sc
                ptag = ptag or "ps_sc"
                ps = pool.tile([128, S], F32, tag=ptag,
                               name=f"op{_rep}_{sb}_{acc}")
                for ch in range(NCH):
                    for ti, (dmat, qk, pa) in enumerate(terms):
                        nc.tensor.matmul(
                            ps[:, ch * 512 : (ch + 1) * 512],
                            lhsT=dmat[:, pa * S + sb * 128 : pa * S + (sb + 1) * 128],
                            rhs=wn_all[:, (qk * NPAIR + pa) * D + ch * 512 :
                                       (qk * NPAIR + pa) * D + ch * 512 + 512],
                            start=(ti == 0),
                            stop=(ti == len(terms) - 1),
                        )
                if acc:
                    nc.vector.tensor_tensor(
                        go_all[:, sb * S : (sb + 1) * S],
                        go_all[:, sb * S : (sb + 1) * S], ps[:], ADD)
                else:
                    nc.vector.tensor_copy(go_all[:, sb * S : (sb + 1) * S], ps[:])

            def pair_end(p):
                """1/Z rows [1, S] for the dQ rescale + dK evacuation."""
                zsum2, dk_ps = pairs[p]["zsum2"], pairs[p]["dk_ps"]
                nc.vector.tensor_copy(dkt2[:, p * S : (p + 1) * S], dk_ps[:])
                zinv2 = sb2.tile([128, 16], F32, tag="zinv2")
                nc.vector.reciprocal(zinv2[:], zsum2[:])
                zt_ps = ps_sc.tile([128, 128], F32, tag="ps_sc",
                                   name=f"ztp{_rep}_{p}")
                nc.tensor.transpose(zt_ps[0:16, 0:128], zinv2[:], ident[:])
                nc.vector.tensor_copy(ztsb[:], zt_ps[0:16, 0:128])
                for a in range(2):
                    nc.scalar.dma_start(
                        zrows[p * 2 + a][:].rearrange("p (b c) -> p b c", b=NQ),
                        ztsb[a * NQ : (a + 1) * NQ, :],
                    )

            def alloc_pair(p):
                d = {}
                d["qt2"] = sb2.tile([128, S], F16, tag="qt2", name=f"qt2_{_rep}_{p}")
                d["kt2"] = sb2.tile([128, S], F16, tag="kt2", name=f"kt2_{_rep}_{p}")
                d["qraw"] = sb2.tile([128, NQ * 128], F16, tag="qraw",
                                     name=f"qraw_{_rep}_{p}")
                d["k2n"] = sb2.tile([128, NQ * 128], F16, tag="k2n",
                                    name=f"k2n_{_rep}_{p}")
                d["zsum2"] = sb2.tile([128, 16], F32, tag="zsum2",
                                      name=f"zsum2_{_rep}_{p}")
                d["P_all"] = pp.tile([128, 2 * NQ * S], F16, tag="P",
                                     name=f"P{_rep}_{p}")
                d["PT_all"] = pp.tile([128, 2 * NQ * S], F16, tag="PT",
                                      name=f"PT{_rep}_{p}")
                d["PT_r"] = d["PT_all"][:].rearrange("p (a j s) -> p a j s",
                                                     a=2, j=NQ)
                return d

            def qraw_k2n(p):
                nc.sync.dma_start_transpose(
                    pairs[p]["qraw"][:].rearrange("p (j z) -> p j z", j=NQ),
                    pairs[p]["qt2"][:])
                nc.sync.dma_start_transpose(
                    pairs[p]["k2n"][:].rearrange("p (j z) -> p j z", j=NQ),
                    pairs[p]["kt2"][:])

            # ================= schedule =================
            pairs.append(alloc_pair(0))
            pairs.append(alloc_pair(1))

            # pair-0 projections up front
            for qk in range(2):
                ps = ps_d.tile([128, S], F32, tag="ps_d", name=f"pj{_rep}_0_{qk}")
                proj_chunk(0, qk, ps, list(range(ND)),
                           pairs[0]["qt2"] if qk == 0 else pairs[0]["kt2"])
            qraw_k2n(0)

            # pair-0 loop; pair-1 projections ride along (4 MMs per slot)
            pj_ps = {}
            dk0 = pairs[0]["dk_ps"] = ps_d.tile([128, S], F32, tag="ps_d",
                                                name=f"dk{_rep}_0")
            for i in range(NQ + 1):
                if i < NQ:
                    scores_exp(0, i)
                if i < 4:
                    if i == 0:
                        pj_ps[0] = ps_d.tile([128, S], F32, tag="ps_d",
                                             name=f"pj{_rep}_1_0")
                    proj_chunk(1, 0, pj_ps[0], [2 * i, 2 * i + 1],
                               pairs[1]["qt2"])
                elif i < NQ:
                    if i == 4:
                        pj_ps[1] = ps_d.tile([128, S], F32, tag="ps_d",
                                             name=f"pj{_rep}_1_1")
                    proj_chunk(1, 1, pj_ps[1], [2 * (i - 4), 2 * (i - 4) + 1],
                               pairs[1]["kt2"])
                if i >= 1:
                    dk_step(0, i - 1)
            pair_end(0)
            qraw_k2n(1)

            # pair-1 loop; pair-0's dQ burst covers the transition, pair-0's
            # output-projection terms ride along
            emit_dq_burst(0, "b1")
            pairs[1]["dk_ps"] = ps_d.tile([128, S], F32, tag="ps_d",
                                          name=f"dk{_rep}_1")
            for i in range(NQ + 1):
                if i < NQ:
                    scores_exp(1, i)
                if i >= 1:
                    dk_step(1, i - 1)
                if i < NQ:
                    out_chain(i, [(dqt2, 0, 0), (dkt2, 1, 0)], acc=False,
                              pool=ps_d, ptag="ps_d")
            pair_end(1)

            # tail: pair-1 dQ + the pair-1 output-projection terms.
            # dk-half chains first (ready at pair_end), dq-half after the
            # rescale; evacuations split DVE/GpSimd; gout DMA chunked.
            emit_dq_burst(1, "tail")
            for sb in range(NQ):
                if sb % 2 == 0:
                    out_chain(sb, [(dkt2, 1, 1), (dqt2, 0, 1)], acc=True)
                else:
                    out_chain(sb, [(dkt2, 1, 1), (dqt2, 0, 1)], acc=True,
                              pool=ps_d, ptag="ps_d")
                nc.sync.dma_start(
                    gout[sb * 128 : (sb + 1) * 128, :],
                    go_all[:, sb * S : (sb + 1) * S],
                )

    nc.compile()
    return nc


def core_inputs(x, wq, wk, core):
    """Per-core input arrays (host-side shard/layout prep, all cheap)."""
    b = core // 4
    h0 = 4 * (core % 4)
    xt = np.ascontiguousarray(x[b].T).astype(np.float16)
    wq4 = wq[h0 : h0 + 4].reshape(NPAIR, 128, D)
    wk4 = wk[h0 : h0 + 4].reshape(NPAIR, 128, D)
    wn = np.concatenate(
        [wq4.reshape(NPAIR * 128, D), wk4.reshape(NPAIR * 128, D)]
    ).astype(np.float16)
    # wt[p, (qk, pair, dt, z2)] = w[qk][pair, z2, dt*128 + p]
    wstack = np.stack([wq4, wk4])                    # [qk, pair, z2, d]
    wt = (
        wstack.reshape(2, NPAIR, 128, ND, 128)       # [qk, pair, z2, dt, p]
        .transpose(4, 0, 1, 3, 2)                    # [p, qk, pair, dt, z2]
        .reshape(128, 2 * NPAIR * ND * 128)
    )
    wt = np.ascontiguousarray(wt).astype(np.float16)
    return {"xt": xt, "wn": wn, "wt": wt}


def combine(gouts):
    """Host unshard: sum the 4 positive partials per batch, apply pos term."""
    pos = np.linspace(-0.5, 0.5, S, dtype=np.float32)[:, None] * np.float32(POS_SCALE)
    out = np.empty((B, S, D), np.float32)
    for b in range(B):
        acc = np.asarray(gouts[4 * b], np.float32)
        for c in range(4 * b + 1, 4 * b + 4):
            acc += np.asarray(gouts[c], np.float32)
        out[b] = pos - acc
    return out


def _build_persistent(nc):
    """One-time jitted sharded callable over the Bass NEFF (no per-call
    retracing; outputs are fully written by the kernel so no donation)."""
    import jax
    import numpy as _np
    from jax.experimental.shard_map import shard_map
    from jax.sharding import Mesh, NamedSharding, PartitionSpec

    import concourse.mybir as mybir
    from concourse import bass2jax

    bass2jax.install_neuronx_cc_hook()
    partition_name = nc.partition_id_tensor.name if nc.partition_id_tensor else None
    in_names, out_names, out_avals = [], [], []
    for alloc in nc.m.functions[0].allocations:
        if not isinstance(alloc, mybir.MemoryLocationSet):
            continue
        name = alloc.memorylocations[0].name
        if alloc.kind == "ExternalInput":
            if name != partition_name:
                in_names.append(name)
        elif alloc.kind == "ExternalOutput":
            out_names.append(name)
            out_avals.append(
                jax.core.ShapedArray(tuple(alloc.tensor_shape), mybir.dt.np(alloc.dtype))
            )
    n_params = len(in_names)
    all_in_names = list(in_names) + out_names
    if partition_name is not None:
        all_in_names.append(partition_name)

    def _body(*args):
        operands = list(args)
        if partition_name is not None:
            operands.append(bass2jax.partition_id_tensor())
        return tuple(
            bass2jax._bass_exec_p.bind(
                *operands,
                out_avals=tuple(out_avals),
                in_names=tuple(all_in_names),
                out_names=tuple(out_names),
                lowering_input_output_aliases=(),
                sim_require_finite=True,
                sim_require_nnan=True,
                nc=nc,
            )
        )

    devices = jax.devices()[:N_CORES]
    mesh = Mesh(_np.asarray(devices), ("core",))
    spec = PartitionSpec("core")
    sharded = jax.jit(
        shard_map(
            _body,
            mesh=mesh,
            in_specs=(spec,) * (n_params + len(out_names)),
            out_specs=(spec,) * len(out_names),
            check_rep=False,
        ),
        keep_unused=True,
    )
    sh = NamedSharding(mesh, spec)
    zeros = [
        jax.device_put(
            _np.zeros((N_CORES * a.shape[0],) + a.shape[1:], a.dtype), sh
        )
        for a in out_avals
    ]
    return {
        "sharded": sharded,
        "in_names": in_names,
        "out_names": out_names,
        "out_avals": out_avals,
        "sh": sh,
        "zeros": zeros,
        "jax": jax,
    }


def kernel(x, wq, wk):
    x = np.asarray(x, np.float32)
    wq = np.asarray(wq, np.float32)
    wk = np.asarray(wk, np.float32)
    if "nc" not in _CACHE:
        _CACHE["nc"] = build_nc()
    nc = _CACHE["nc"]
    if "pc" not in _CACHE:
        _CACHE["pc"] = _build_persistent(nc)
    pc = _CACHE["pc"]
    jax = pc["jax"]

    in_maps = [core_inputs(x, wq, wk, c) for c in range(N_CORES)]
    concat_in = [
        jax.device_put(
            np.concatenate([np.asarray(m[nm]) for m in in_maps], axis=0), pc["sh"]
        )
        for nm in pc["in_names"]
    ]
    outs = pc["sharded"](*concat_in, *pc["zeros"])
    g = np.asarray(outs[pc["out_names"].index("gout")])
    return combine(g.reshape(N_CORES, S, D))
